# revision 42
# baseline (speedup 1.0000x reference)
"""Trainium2 Bass kernel for nn_GroupedLossWithIndexMap.

Reference computation (per batch item b, N=65536 rows, C_old=128, C_new=16):
    probs   = softmax(inputs[b], axis=-1)            # [N, 128]
    grouped = probs @ GROUP_MAT                      # [N, 16] (8 contiguous cols per group)
    avg     = mean(grouped, axis=0)                  # [16]
    loss_b  = KL(softmax(targets[b]/100) || softmax(avg)) / 16
    out     = mean_b(loss_b)

Key identity: grouping+mean commute, so each core only needs
    colsum[c] = sum_n exp(x[n,c]) / rowsum[n]        # [128]
and the rest is trivial scalar math done on host.

Device kernel (per core, one batch item, data parallel over 8 cores):
  - groups of kk rows/partition; each group is one contiguous DMA into a
    [128, kk*128] SBUF tile (partition p holds kk consecutive rows).
  - ACT: exp (f32 -> bf16).
  - DVE: row sums (bf16 halving adds + reduce), reciprocal.
  - PE : per-group matmuls psum[m, m*C] += r^T @ exp accumulated over two
         independent chains: A (bulk stream, taper ending in a kk=2 group so
         the post-last-byte chain is short) and B (covers the LAST rows of x
         but streams right after the first group, retiring early).
  - out: one [4, 6*C] f32 tile -> DRAM; host sums the diagonal blocks.

Latency structure (measured): ~2.4us prologue (a raw pre-barrier head-start
DMA hides most of it), ~84us input stream at DMA line rate, ~5us tail chain,
then a fixed ~7us NRT postamble (253 semaphore clears) that the measured
window includes.  TileContext's exit drain/barriers/clears are replaced by a
semaphore range-clear at ENTRY (see _patch_tile_epilogue) so the postamble
starts as soon as the output DMA is issued rather than after its HBM
receipt.
"""

import numpy as np

B = 8
N = 65536
C = 128
G = 16
P = 128
EPS = 1e-8

MA = 4          # matmul chunk block for bulk chain A
MBL = 2         # matmul chunk block for chain B

# Group schedules (rows_per_partition).  Chain A streams first (bulk plus a
# taper ending in a tiny group so the post-last-byte dependency chain is
# short); chain B covers the LAST rows of x but is streamed right after the
# first group, so its whole pipeline (exp, rowsums, matmuls, PSUM copy)
# retires early and the kernel tail is only A's final tiny group.
SPECS_A = [32] * 14 + [16, 16, 8, 8, 4, 2, 2]
SPECS_B = [4, 2, 2]
assert (sum(SPECS_A) + sum(SPECS_B)) * P == N

_compiled = None

# Semaphores cleared at kernel entry (covers every sem TileContext
# lazily allocates — asserted at build time in the patched epilogue).
# 155 is reserved for the raw head-start DMA's semaphore.
_RAWSEM = 155
_PRECLEAR = range(156, 177)


def _patch_tile_epilogue(tile):
    """Drop TileContext's end-of-kernel drain+barrier+clear entirely.

    The exit drain existed so the output DMA's completion increment lands
    before the NRT postamble zeroes every semaphore; without it the +16
    receipt can arrive post-clear and leak into the next execution of the
    same NEFF, making one DMA-lane wait pass a transfer early.  We instead
    neutralize the leak at the START of the kernel (gpsimd range-clear of
    Tile's sems + sem-only barrier, emitted in _build before TileContext),
    which is ~5x cheaper than delaying the postamble behind the out-DMA's
    HBM receipt round-trip."""
    if getattr(tile.TileContext, "_fast_epilogue", False):
        return

    def _drain_and_barrier(self, tick_clock, wait_clock):
        nums = [s.num for s in self.sems.allocated().values()]
        assert all(n in _PRECLEAR for n in nums), nums
        popped = self.nc._tile_sem_poison_stack.pop()
        assert popped is self._sem_poison

    tile.TileContext._drain_and_barrier = _drain_and_barrier
    tile.TileContext._fast_epilogue = True


def _build(specs_a=SPECS_A, specs_b=SPECS_B):
    import concourse.bacc as bacc
    import concourse.bass as bass
    import concourse.tile as tile
    from concourse import mybir

    _patch_tile_epilogue(tile)

    f32 = mybir.dt.float32
    bf16 = mybir.dt.bfloat16

    n = P * (sum(specs_a) + sum(specs_b))

    # Emission order = DMA stream order: A's first bulk group (head start),
    # then all of B (which covers the last rows of x), then the rest of A.
    rows_a = []
    r = 0
    for kk in specs_a:
        rows_a.append(r)
        r += P * kk
    rows_b = []
    r = P * sum(specs_a)
    for kk in specs_b:
        rows_b.append(r)
        r += P * kk
    order = [(specs_a[0], 0, rows_a[0])] \
        + [(kk, 1, r0) for kk, r0 in zip(specs_b, rows_b)] \
        + [(kk, 0, r0) for kk, r0 in zip(specs_a[1:], rows_a[1:])]

    nc = bacc.Bacc(
        "TRN2",
        target_bir_lowering=False,
        debug=False,
        num_devices=B,
    )

    x = nc.dram_tensor("x", [n, C], f32, kind="ExternalInput")
    # [4, (MA + MBL) * C]: first MA*C cols are chain A's psum block, the
    # next MBL*C cols (partitions 0:MBL) are chain B's.
    OC = (MA + MBL) * C
    colsum = nc.dram_tensor("colsum", [MA, OC], f32, kind="ExternalOutput")

    # Head start: issue group 0's DMA before the entry barrier so its 2 MB
    # streams while the prologue (constant memsets, barrier, ACT table load)
    # is still running.  It uses a dedicated raw semaphore and a raw SBUF
    # region, both outside Tile's management.
    kk0 = specs_a[0]
    rawsem = nc.alloc_semaphore("x0_dma")
    assert rawsem.num == _RAWSEM, rawsem.num
    x0 = nc.alloc_sbuf_tensor("x0", [P, kk0 * C], f32)
    src0 = x.ap()[0 : P * kk0, :].rearrange("(p k) c -> p (k c)", p=P, k=kk0)
    # Issue from ACT's HWDGE ring: ACT reaches this instruction ~0.7us
    # earlier than Sync would (shorter NRT preamble), and ACT has nothing
    # else to do until this transfer lands anyway.
    nc.scalar.dma_start(out=x0.ap(), in_=src0).then_inc(rawsem, 16)

    # Entry-side reset of Tile's semaphores (see _patch_tile_epilogue): any
    # DMA-lane increment that landed after the previous run's postamble
    # clears is zeroed here, before any engine can wait on (or bump) them.
    nc.gpsimd.dma_reset(_PRECLEAR)
    nc.gpsimd.sem_clear(_PRECLEAR)
    nc.all_engine_barrier(sem_only=True)
    # Gate ACT on the head-start DMA here, in the root block: ACT's program
    # order puts this before the tile body's first exp, and keeping it
    # outside TileContext hides it from the scheduler's deadlock simulation
    # (which replays only the tile block, where nothing increments rawsem).
    nc.scalar.wait_ge(rawsem, 16)

    nmmA = sum((kk + MA - 1) // MA for kk in specs_a)
    nmmB = sum((kk + MBL - 1) // MBL for kk in specs_b)
    lastB = max(i for i, (_, ch, _) in enumerate(order) if ch == 1)

    with tile.TileContext(nc) as tc:
        with (
            tc.tile_pool(name="xin", bufs=6) as xpool,
            tc.tile_pool(name="exp", bufs=6) as epool,
            tc.tile_pool(name="half", bufs=3) as hpool,
            tc.tile_pool(name="small", bufs=8) as spool,
            tc.tile_pool(name="out", bufs=1) as opool,
            tc.tile_pool(name="psum", bufs=1, space="PSUM") as ppool,
        ):
            psA = ppool.tile([MA, MA * C], f32, tag="psA")
            psB = ppool.tile([MBL, MBL * C], f32, tag="psB")
            ot = opool.tile([MA, OC], f32)
            mmiA = 0
            mmiB = 0
            for g, (kk, ch, row0) in enumerate(order):
                # Small groups get their own pool tags: tags rotate slots
                # independently, so (a) tiny tiles never consume the bulk
                # pools' byte-deep rotation slots, and (b) the taper's
                # end-of-stream DMA issues never wait on bulk exp progress
                # (which showed up as a ~1us DMA gap before the last bytes).
                sfx = "b" if ch == 1 else ""
                et = epool.tile([P, kk * C], bf16, tag="e" + sfx)
                if g == 0:
                    # group 0 was DMA'd into the raw head-start region before
                    # the entry barrier; ACT already waited on its semaphore
                    # in the root block just before branching into this body.
                    nc.scalar.activation(
                        et[:], x0.ap(), mybir.ActivationFunctionType.Exp
                    )
                else:
                    # partition p holds rows row0 + p*kk + [0, kk)
                    src = (
                        x.ap()[row0 : row0 + P * kk, :]
                        .rearrange("(p k) c -> p (k c)", p=P, k=kk)
                    )
                    xt = xpool.tile([P, kk * C], f32, tag="x" + sfx)
                    nc.sync.dma_start(out=xt[:], in_=src)
                    nc.scalar.activation(
                        et[:], xt[:], mybir.ActivationFunctionType.Exp
                    )

                e3 = et[:].rearrange("p (k c) -> p k c", c=C)
                st = spool.tile([P, kk], f32, tag="s" + sfx)
                if kk <= 4:
                    # tiny group: single reduce beats the tree's op count
                    nc.vector.reduce_sum(st[:], e3, axis=mybir.AxisListType.X)
                else:
                    # bf16 halving adds run at 2x DVE rate; the final 1x-rate
                    # reduce only sees 8 columns per row
                    at = hpool.tile([P, kk * 64], bf16, tag="a")
                    a3 = at[:].rearrange("p (k c) -> p k c", c=64)
                    nc.vector.tensor_add(a3, e3[:, :, 0:64], e3[:, :, 64:128])
                    bt = hpool.tile([P, kk * 32], bf16, tag="b")
                    b3 = bt[:].rearrange("p (k c) -> p k c", c=32)
                    nc.vector.tensor_add(b3, a3[:, :, 0:32], a3[:, :, 32:64])
                    nc.vector.reduce_sum(st[:], b3, axis=mybir.AxisListType.X)
                rb = spool.tile([P, kk], bf16, tag="rb" + sfx)
                with nc.allow_low_precision("bf16 reciprocal weights"):
                    nc.vector.reciprocal(rb[:], st[:])

                if ch == 0:
                    for k0 in range(0, kk, MA):
                        m = min(MA, kk - k0)
                        nc.tensor.matmul(
                            psA[0:m, 0 : m * C],
                            rb[:, k0 : k0 + m],
                            et[:, k0 * C : (k0 + m) * C],
                            start=(mmiA == 0),
                            stop=(mmiA == nmmA - 1),
                        )
                        mmiA += 1
                else:
                    for k0 in range(0, kk, MBL):
                        m = min(MBL, kk - k0)
                        nc.tensor.matmul(
                            psB[0:m, 0 : m * C],
                            rb[:, k0 : k0 + m],
                            et[:, k0 * C : (k0 + m) * C],
                            start=(mmiB == 0),
                            stop=(mmiB == nmmB - 1),
                        )
                        mmiB += 1

                # Chain B finishes within the first few groups; its PSUM copy
                # (DVE) retires long before the tail.
                if g == lastB:
                    nc.vector.tensor_copy(ot[0:MBL, MA * C : OC], psB[:])
            assert mmiA == nmmA and mmiB == nmmB

            # Chain A's copy is the only end-of-stream work; split it in
            # half across DVE and ACT so the two run concurrently (each
            # copy's completion also pays its engine's pipe-drain, so one
            # big copy costs ~1.1us serial vs ~0.65us for the halves).  One
            # DMA ships both chains.
            half = MA * C // 2
            nc.vector.tensor_copy(ot[:, 0:half], psA[:, 0:half])
            nc.scalar.copy(ot[:, half : MA * C], psA[:, half : MA * C])
            nc.sync.dma_start(out=colsum[:], in_=ot[:])

    nc.compile()
    return nc


def _get_compiled():
    global _compiled
    if _compiled is None:
        _compiled = _build()
    return _compiled


def _run_device(inputs: np.ndarray, trace: bool = False, **kwargs):
    from concourse.bass_utils import run_bass_kernel_spmd

    nc = _get_compiled()
    in_maps = [
        {"x": np.ascontiguousarray(inputs[i], dtype=np.float32)} for i in range(B)
    ]
    res = run_bass_kernel_spmd(nc, in_maps, list(range(B)), trace=trace, **kwargs)
    colsums = []
    for i in range(B):
        arr = (
            np.asarray(res.results[i]["colsum"], dtype=np.float64)
            .reshape(MA, MA + MBL, C)
        )
        cs = arr[np.arange(MA), np.arange(MA)].sum(axis=0)        # chain A diag
        cs += arr[np.arange(MBL), MA + np.arange(MBL)].sum(axis=0)  # chain B diag
        colsums.append(cs)
    return np.stack(colsums), res


def _finish_host(colsums: np.ndarray, targets: np.ndarray) -> np.ndarray:
    # colsums: [B, 128] float; targets: [B, 16]
    cs = colsums.astype(np.float64)
    avg = cs.reshape(B, G, C // G).sum(axis=-1) / N          # [B, 16]
    # softmax(avg)
    a = avg - avg.max(axis=-1, keepdims=True)
    p = np.exp(a)
    p /= p.sum(axis=-1, keepdims=True)
    # softmax(targets / 100)
    t = targets.astype(np.float64) / 100.0
    t = t - t.max(axis=-1, keepdims=True)
    t = np.exp(t)
    t /= t.sum(axis=-1, keepdims=True)
    log_p = np.log(p + EPS)
    kl = (t * (np.log(t) - log_p)).sum(axis=-1) / G          # [B]
    return np.float32(kl.mean())


def kernel(inputs: np.ndarray, targets: np.ndarray) -> np.ndarray:
    colsums, _ = _run_device(np.asarray(inputs))
    return _finish_host(colsums, np.asarray(targets))


# revision 43
# speedup vs baseline: 1.0066x; 1.0066x over previous
"""Trainium2 Bass kernel for nn_GroupedLossWithIndexMap.

Reference computation (per batch item b, N=65536 rows, C_old=128, C_new=16):
    probs   = softmax(inputs[b], axis=-1)            # [N, 128]
    grouped = probs @ GROUP_MAT                      # [N, 16] (8 contiguous cols per group)
    avg     = mean(grouped, axis=0)                  # [16]
    loss_b  = KL(softmax(targets[b]/100) || softmax(avg)) / 16
    out     = mean_b(loss_b)

Key identity: grouping+mean commute, so each core only needs
    colsum[c] = sum_n exp(x[n,c]) / rowsum[n]        # [128]
and the rest is trivial scalar math done on host.

Device kernel (per core, one batch item, data parallel over 8 cores):
  - groups of kk rows/partition; each group is one contiguous DMA into a
    [128, kk*128] SBUF tile (partition p holds kk consecutive rows).
  - ACT: exp (f32 -> bf16).
  - DVE: row sums (bf16 halving adds + reduce), reciprocal.
  - PE : per-group matmuls psum[m, m*C] += r^T @ exp accumulated over two
         independent chains: A (bulk stream, taper ending in a kk=2 group so
         the post-last-byte chain is short) and B (covers the LAST rows of x
         but streams right after the first group, retiring early).
  - out: one [4, 6*C] f32 tile -> DRAM; host sums the diagonal blocks.

Latency structure (measured): ~2.4us prologue (a raw pre-barrier head-start
DMA hides most of it), ~84us input stream at DMA line rate, ~5us tail chain,
then a fixed ~7us NRT postamble (253 semaphore clears) that the measured
window includes.  TileContext's exit drain/barriers/clears are replaced by a
semaphore range-clear at ENTRY (see _patch_tile_epilogue) so the postamble
starts as soon as the output DMA is issued rather than after its HBM
receipt.
"""

import numpy as np

B = 8
N = 65536
C = 128
G = 16
P = 128
EPS = 1e-8

MA = 4          # matmul chunk block for bulk chain A
MBL = 2         # matmul chunk block for chain B

# Group schedules (rows_per_partition).  Chain A streams first (bulk plus a
# taper ending in a tiny group so the post-last-byte dependency chain is
# short); chain B covers the LAST rows of x but is streamed right after the
# first group, so its whole pipeline (exp, rowsums, matmuls, PSUM copy)
# retires early and the kernel tail is only A's final tiny group.
SPECS_A = [32] * 14 + [16, 16, 8, 8, 4, 2, 2]
SPECS_B = [4, 2, 2]
assert (sum(SPECS_A) + sum(SPECS_B)) * P == N

_compiled = None

# Semaphores cleared at kernel entry (covers every sem TileContext
# lazily allocates — asserted at build time in the patched epilogue).
# 155 is reserved for the raw head-start DMA's semaphore.
_RAWSEM = 155
_PRECLEAR = range(156, 177)


def _patch_tile_epilogue(tile):
    """Drop TileContext's end-of-kernel drain+barrier+clear entirely.

    The exit drain existed so the output DMA's completion increment lands
    before the NRT postamble zeroes every semaphore; without it the +16
    receipt can arrive post-clear and leak into the next execution of the
    same NEFF, making one DMA-lane wait pass a transfer early.  We instead
    neutralize the leak at the START of the kernel (gpsimd range-clear of
    Tile's sems + sem-only barrier, emitted in _build before TileContext),
    which is ~5x cheaper than delaying the postamble behind the out-DMA's
    HBM receipt round-trip."""
    if getattr(tile.TileContext, "_fast_epilogue", False):
        return

    def _drain_and_barrier(self, tick_clock, wait_clock):
        nums = [s.num for s in self.sems.allocated().values()]
        assert all(n in _PRECLEAR for n in nums), nums
        popped = self.nc._tile_sem_poison_stack.pop()
        assert popped is self._sem_poison

    tile.TileContext._drain_and_barrier = _drain_and_barrier
    tile.TileContext._fast_epilogue = True


def _build(specs_a=SPECS_A, specs_b=SPECS_B):
    import concourse.bacc as bacc
    import concourse.bass as bass
    import concourse.tile as tile
    from concourse import mybir

    _patch_tile_epilogue(tile)

    f32 = mybir.dt.float32
    bf16 = mybir.dt.bfloat16

    n = P * (sum(specs_a) + sum(specs_b))

    # Emission order = DMA stream order: A's first bulk group (head start),
    # then all of B (which covers the last rows of x), then the rest of A.
    rows_a = []
    r = 0
    for kk in specs_a:
        rows_a.append(r)
        r += P * kk
    rows_b = []
    r = P * sum(specs_a)
    for kk in specs_b:
        rows_b.append(r)
        r += P * kk
    order = [(specs_a[0], 0, rows_a[0])] \
        + [(kk, 1, r0) for kk, r0 in zip(specs_b, rows_b)] \
        + [(kk, 0, r0) for kk, r0 in zip(specs_a[1:], rows_a[1:])]

    nc = bacc.Bacc(
        "TRN2",
        target_bir_lowering=False,
        debug=False,
        num_devices=B,
    )

    x = nc.dram_tensor("x", [n, C], f32, kind="ExternalInput")
    # [4, (MA + MBL) * C]: first MA*C cols are chain A's psum block, the
    # next MBL*C cols (partitions 0:MBL) are chain B's.
    OC = (MA + MBL) * C
    colsum = nc.dram_tensor("colsum", [MA, OC], f32, kind="ExternalOutput")

    # Head start: issue group 0's DMA before the entry barrier so its 2 MB
    # streams while the prologue (constant memsets, barrier, ACT table load)
    # is still running.  It uses a dedicated raw semaphore and a raw SBUF
    # region, both outside Tile's management.
    kk0 = specs_a[0]
    rawsem = nc.alloc_semaphore("x0_dma")
    assert rawsem.num == _RAWSEM, rawsem.num
    x0 = nc.alloc_sbuf_tensor("x0", [P, kk0 * C], f32)
    src0 = x.ap()[0 : P * kk0, :].rearrange("(p k) c -> p (k c)", p=P, k=kk0)
    # Issue from ACT's HWDGE ring: ACT reaches this instruction ~0.7us
    # earlier than Sync would (shorter NRT preamble), and ACT has nothing
    # else to do until this transfer lands anyway.
    nc.scalar.dma_start(out=x0.ap(), in_=src0).then_inc(rawsem, 16)

    # Entry-side reset of Tile's semaphores (see _patch_tile_epilogue): any
    # DMA-lane increment that landed after the previous run's postamble
    # clears is zeroed here, before any engine can wait on (or bump) them.
    nc.gpsimd.dma_reset(_PRECLEAR)
    nc.gpsimd.sem_clear(_PRECLEAR)
    nc.all_engine_barrier(sem_only=True)
    # Gate ACT on the head-start DMA here, in the root block: ACT's program
    # order puts this before the tile body's first exp, and keeping it
    # outside TileContext hides it from the scheduler's deadlock simulation
    # (which replays only the tile block, where nothing increments rawsem).
    nc.scalar.wait_ge(rawsem, 16)

    nmmA = sum((kk + MA - 1) // MA for kk in specs_a)
    nmmB = sum((kk + MBL - 1) // MBL for kk in specs_b)
    lastB = max(i for i, (_, ch, _) in enumerate(order) if ch == 1)

    with tile.TileContext(nc) as tc:
        with (
            tc.tile_pool(name="xin", bufs=6) as xpool,
            tc.tile_pool(name="exp", bufs=6) as epool,
            tc.tile_pool(name="half", bufs=3) as hpool,
            tc.tile_pool(name="small", bufs=8) as spool,
            tc.tile_pool(name="out", bufs=1) as opool,
            tc.tile_pool(name="psum", bufs=1, space="PSUM") as ppool,
        ):
            psA = ppool.tile([MA, MA * C], f32, tag="psA")
            psB = ppool.tile([MBL, MBL * C], f32, tag="psB")
            ot = opool.tile([MA, OC], f32)
            mmiA = 0
            mmiB = 0
            for g, (kk, ch, row0) in enumerate(order):
                # Small groups get their own pool tags: tags rotate slots
                # independently, so (a) tiny tiles never consume the bulk
                # pools' byte-deep rotation slots, and (b) the taper's
                # end-of-stream DMA issues never wait on bulk exp progress
                # (which showed up as a ~1us DMA gap before the last bytes).
                sfx = "b" if ch == 1 else ""
                et = epool.tile([P, kk * C], bf16, tag="e" + sfx)
                if g == 0:
                    # group 0 was DMA'd into the raw head-start region before
                    # the entry barrier; ACT already waited on its semaphore
                    # in the root block just before branching into this body.
                    nc.scalar.activation(
                        et[:], x0.ap(), mybir.ActivationFunctionType.Exp
                    )
                else:
                    # partition p holds rows row0 + p*kk + [0, kk)
                    src = (
                        x.ap()[row0 : row0 + P * kk, :]
                        .rearrange("(p k) c -> p (k c)", p=P, k=kk)
                    )
                    xt = xpool.tile([P, kk * C], f32, tag="x" + sfx)
                    nc.sync.dma_start(out=xt[:], in_=src)
                    nc.scalar.activation(
                        et[:], xt[:], mybir.ActivationFunctionType.Exp
                    )

                e3 = et[:].rearrange("p (k c) -> p k c", c=C)
                st = spool.tile([P, kk], f32, tag="s" + sfx)
                if kk <= 4:
                    # tiny group: single reduce beats the tree's op count
                    nc.vector.reduce_sum(st[:], e3, axis=mybir.AxisListType.X)
                else:
                    # bf16 halving adds run at 2x DVE rate; the final 1x-rate
                    # reduce only sees 8 columns per row
                    at = hpool.tile([P, kk * 64], bf16, tag="a")
                    a3 = at[:].rearrange("p (k c) -> p k c", c=64)
                    nc.vector.tensor_add(a3, e3[:, :, 0:64], e3[:, :, 64:128])
                    bt = hpool.tile([P, kk * 32], bf16, tag="b")
                    b3 = bt[:].rearrange("p (k c) -> p k c", c=32)
                    nc.vector.tensor_add(b3, a3[:, :, 0:32], a3[:, :, 32:64])
                    nc.vector.reduce_sum(st[:], b3, axis=mybir.AxisListType.X)
                rb = spool.tile([P, kk], bf16, tag="rb" + sfx)
                with nc.allow_low_precision("bf16 reciprocal weights"):
                    nc.vector.reciprocal(rb[:], st[:])

                if ch == 0:
                    for k0 in range(0, kk, MA):
                        m = min(MA, kk - k0)
                        nc.tensor.matmul(
                            psA[0:m, 0 : m * C],
                            rb[:, k0 : k0 + m],
                            et[:, k0 * C : (k0 + m) * C],
                            start=(mmiA == 0),
                            stop=(mmiA == nmmA - 1),
                        )
                        mmiA += 1
                else:
                    for k0 in range(0, kk, MBL):
                        m = min(MBL, kk - k0)
                        nc.tensor.matmul(
                            psB[0:m, 0 : m * C],
                            rb[:, k0 : k0 + m],
                            et[:, k0 * C : (k0 + m) * C],
                            start=(mmiB == 0),
                            stop=(mmiB == nmmB - 1),
                        )
                        mmiB += 1

                # Chain B finishes within the first few groups; its PSUM copy
                # (DVE) retires long before the tail.
                if g == lastB:
                    nc.vector.tensor_copy(ot[0:MBL, MA * C : OC], psB[:])
            assert mmiA == nmmA and mmiB == nmmB

            # Chain A's copy is the only end-of-stream work; DVE is idle by
            # then.  (Splitting it across DVE+ACT was tried and is WORSE:
            # Tile's write-write tracking on ot serializes the two copies.)
            # One DMA ships both chains.
            nc.vector.tensor_copy(ot[:, 0 : MA * C], psA[:])
            nc.sync.dma_start(out=colsum[:], in_=ot[:])

    nc.compile()
    return nc


def _get_compiled():
    global _compiled
    if _compiled is None:
        _compiled = _build()
    return _compiled


def _run_device(inputs: np.ndarray, trace: bool = False, **kwargs):
    from concourse.bass_utils import run_bass_kernel_spmd

    nc = _get_compiled()
    in_maps = [
        {"x": np.ascontiguousarray(inputs[i], dtype=np.float32)} for i in range(B)
    ]
    res = run_bass_kernel_spmd(nc, in_maps, list(range(B)), trace=trace, **kwargs)
    colsums = []
    for i in range(B):
        arr = (
            np.asarray(res.results[i]["colsum"], dtype=np.float64)
            .reshape(MA, MA + MBL, C)
        )
        cs = arr[np.arange(MA), np.arange(MA)].sum(axis=0)        # chain A diag
        cs += arr[np.arange(MBL), MA + np.arange(MBL)].sum(axis=0)  # chain B diag
        colsums.append(cs)
    return np.stack(colsums), res


def _finish_host(colsums: np.ndarray, targets: np.ndarray) -> np.ndarray:
    # colsums: [B, 128] float; targets: [B, 16]
    cs = colsums.astype(np.float64)
    avg = cs.reshape(B, G, C // G).sum(axis=-1) / N          # [B, 16]
    # softmax(avg)
    a = avg - avg.max(axis=-1, keepdims=True)
    p = np.exp(a)
    p /= p.sum(axis=-1, keepdims=True)
    # softmax(targets / 100)
    t = targets.astype(np.float64) / 100.0
    t = t - t.max(axis=-1, keepdims=True)
    t = np.exp(t)
    t /= t.sum(axis=-1, keepdims=True)
    log_p = np.log(p + EPS)
    kl = (t * (np.log(t) - log_p)).sum(axis=-1) / G          # [B]
    return np.float32(kl.mean())


def kernel(inputs: np.ndarray, targets: np.ndarray) -> np.ndarray:
    colsums, _ = _run_device(np.asarray(inputs))
    return _finish_host(colsums, np.asarray(targets))
